# revision 2
# baseline (speedup 1.0000x reference)
"""Trainium2 Bass kernel for nn_AdjacencyMaskedNet.

Reference math (N=4096, I=512, O=512, O_=8 groups, H=2048, GROUP=64):
    for each group g: h_g = relu((x * A_mask[:, g]) @ W1 + b1)
                      y_g = h_g @ W2 + b2
    out[n, j] = y_{col_idx[j]}[n, j]

With the canonical inputs, A_mask[:, g] is the prefix mask over the first
64*(g+1) inputs and col_idx = arange(512) // 64.  Two structural wins:

  1. mm1 is computed *incrementally*: a_g = a_{g-1} + x[:, 64g:64g+64] @
     W1[64g:64g+64, :].  One full-matmul worth of FLOPs total instead of 8.
     The running sum lives in a PSUM bank (per H-block of 128); after each
     K=64 increment the snapshot is relu'd out to SBUF.
  2. mm2 only needs the 64 output columns belonging to each group:
     out[:, cols_g] = relu(a_g) @ W2[:, cols_g], an 8x reduction.

Sharding: data-parallel over batch, 512 rows per core, 8 cores, no
collectives.  Weights are broadcast; each core computes outT (O, 512) for
its batch shard; host reassembles.

dtypes: mm1 in float32r (full-rate fp32, ~1.5e-4 component error), mm2 in
bf16 (the h tiles are produced in bf16 directly by the relu).  PSUM
accumulation is fp32 throughout.
"""

import sys

if "/opt/trn_rl_repo" not in sys.path:
    sys.path.insert(0, "/opt/trn_rl_repo")

import numpy as np
import ml_dtypes

N, I, O, O_, H = 4096, 512, 512, 8, 2048
GROUP = O // O_  # 64
NCORES = 8
NC = N // NCORES  # 512 batch rows per core
HB = H // 128  # 16 H-blocks
NPAIR = O // 128  # 4 psum banks for mm2 (two 64-col groups per bank)

_CACHE = {}


def _canonical_mask():
    g = np.arange(O_)
    return (np.arange(I)[:, None] < (g[None, :] + 1) * (I // O_)).astype(np.float32)


def _build_program():
    """Build + compile the Bass program once per process."""
    import concourse.tile as tile
    from concourse import bacc, mybir

    f32 = mybir.dt.float32
    f32r = mybir.dt.float32r
    bf16 = mybir.dt.bfloat16
    Relu = mybir.ActivationFunctionType.Relu
    Ident = mybir.ActivationFunctionType.Identity

    nc = bacc.Bacc("TRN2", target_bir_lowering=False, debug=False, num_devices=NCORES)

    xta = nc.dram_tensor("xta", [O_, 65, NC], f32, kind="ExternalInput").ap()
    w1a = nc.dram_tensor("w1a", [O_, 65, H], f32, kind="ExternalInput").ap()
    w2 = nc.dram_tensor("w2", [H, O], bf16, kind="ExternalInput").ap()
    b2t = nc.dram_tensor("b2t", [128, NPAIR], f32, kind="ExternalInput").ap()
    ot = nc.dram_tensor("ot", [O, NC], f32, kind="ExternalOutput").ap()

    with tile.TileContext(nc) as tc:
        with (
            tc.tile_pool(name="const", bufs=1) as cp,
            tc.tile_pool(name="hpool", bufs=6) as hp,
            tc.tile_pool(name="opool", bufs=1) as op,
            tc.tile_pool(name="ps1", bufs=3, space="PSUM") as ps1,
            tc.tile_pool(name="ps2", bufs=1, space="PSUM") as ps2,
        ):
            # ---- resident inputs ----
            xsb = []
            w1sb = []
            for g in range(O_):
                t = cp.tile([65, NC], f32r, tag=f"xs{g}", name=f"xs{g}")
                nc.sync.dma_start(t[:], xta[g].bitcast(f32r))
                xsb.append(t)
            for g in range(O_):
                t = cp.tile([65, H], f32r, tag=f"w1_{g}", name=f"w1_{g}")
                nc.sync.dma_start(t[:], w1a[g].bitcast(f32r))
                w1sb.append(t)
            w2sb = []
            for k in range(HB):
                t = cp.tile([128, O], bf16, tag=f"w2_{k}", name=f"w2_{k}")
                nc.sync.dma_start(t[:], w2[k * 128 : (k + 1) * 128, :])
                w2sb.append(t)
            b2sb = cp.tile([128, NPAIR], f32, tag="b2")
            nc.sync.dma_start(b2sb[:], b2t[:])

            # zero operands for the has_written-initializing dummy matmuls
            z1 = cp.tile([1, 128], bf16, tag="z1")
            nc.vector.memset(z1[:], 0.0)
            z2 = cp.tile([1, NC], bf16, tag="z2")
            nc.vector.memset(z2[:], 0.0)

            # ---- mm2 accumulators: 4 banks, two 64-col groups packed per bank.
            # start=True on any matmul would clear has_written for the WHOLE
            # bank (nuking the other group), so init each bank once with a
            # zero matmul and accumulate with start=False afterwards.
            mm2ps = []
            for t in range(NPAIR):
                p = ps2.tile([128, NC], f32, tag=f"mm2_{t}", name=f"mm2_{t}")
                nc.tensor.matmul(
                    p[:], z1[:], z2[:], start=True, stop=False, skip_group_check=True
                )
                mm2ps.append(p)

            # ---- main loop: H-blocks in pairs, groups innermost ----
            for hbp in range(HB // 2):
                accs = [ps1.tile([128, NC], f32, tag="acc", name="acc") for _ in range(2)]
                for g in range(O_):
                    for j in range(2):
                        hb = 2 * hbp + j
                        acc = accs[j]
                        nc.tensor.matmul(
                            acc[:],
                            w1sb[g][:, hb * 128 : (hb + 1) * 128],
                            xsb[g][:],
                            start=(g == 0),
                            stop=(g == O_ - 1),
                        )
                        h = hp.tile([128, NC], bf16, tag="h", name="h")
                        if j == 0:
                            nc.scalar.activation(h[:], acc[:], Relu)
                        else:
                            nc.vector.tensor_scalar_max(h[:], acc[:], 0.0)
                        t, half = g // 2, g % 2
                        nc.tensor.matmul(
                            mm2ps[t][64 * half : 64 * half + 64, :],
                            w2sb[hb][:, g * GROUP : (g + 1) * GROUP],
                            h[:],
                            start=False,
                            stop=(hb == HB - 1 and half == 1),
                            skip_group_check=True,
                        )

            # ---- evacuate mm2 accumulators (+b2) and store outT ----
            for t in range(NPAIR):
                os_ = op.tile([128, NC], f32, tag=f"os{t}", name=f"os{t}")
                nc.scalar.activation(os_[:], mm2ps[t][:], Ident, bias=b2sb[:, t : t + 1])
                nc.sync.dma_start(ot[t * 128 : (t + 1) * 128, :], os_[:])

    nc.compile()
    return nc


def _get_program():
    if "nc" not in _CACHE:
        _CACHE["nc"] = _build_program()
    return _CACHE["nc"]


def _run_on_hw(x, W1, b1, W2p, b2p, trace=False, trace_cores=None):
    """Run the bass kernel on 8 cores.  W2p/b2p already column-permuted so
    group g owns contiguous output columns [64g, 64g+64)."""
    from concourse.bass_utils import run_bass_kernel_spmd

    nc = _get_program()

    w1a = np.zeros((O_, 65, H), dtype=np.float32)
    w1a[:, :64, :] = W1.reshape(O_, 64, H)
    w1a[0, 64, :] = b1

    w2bf = np.ascontiguousarray(W2p.astype(ml_dtypes.bfloat16))
    b2t = np.ascontiguousarray(b2p.reshape(NPAIR, 128).T.astype(np.float32))

    in_maps = []
    for c in range(NCORES):
        xs = x[c * NC : (c + 1) * NC, :]  # (NC, I)
        xta = np.zeros((O_, 65, NC), dtype=np.float32)
        xta[:, :64, :] = np.ascontiguousarray(
            xs.T.reshape(O_, 64, NC)
        )  # xta[g, i, n] = x[n, 64g + i]
        xta[0, 64, :] = 1.0
        in_maps.append({"xta": xta, "w1a": w1a, "w2": w2bf, "b2t": b2t})

    kwargs = {}
    if trace:
        kwargs["trace"] = True
        if trace_cores is not None:
            kwargs["trace_cores"] = trace_cores
    res = run_bass_kernel_spmd(nc, in_maps, core_ids=list(range(NCORES)), **kwargs)

    outT = np.stack([res.results[c]["ot"] for c in range(NCORES)])  # (8, O, NC)
    out = np.ascontiguousarray(np.transpose(outT, (0, 2, 1))).reshape(N, O)
    return out, res


def _reference_numpy(x, W1, b1, W2, b2, A_mask, col_idx):
    """Exact fallback for non-canonical adjacency inputs."""
    n = x.shape[0]
    o_ = A_mask.shape[1]
    out = np.empty((n, W2.shape[1]), dtype=np.float32)
    cols_done = np.zeros(W2.shape[1], dtype=bool)
    for g in range(o_):
        cols = np.nonzero(col_idx == g)[0]
        if len(cols) == 0:
            continue
        h = np.maximum(0.0, (x * A_mask[:, g][None, :]) @ W1 + b1)
        out[:, cols] = h @ W2[:, cols] + b2[cols]
        cols_done[cols] = True
    out[:, ~cols_done] = 0.0
    return out


def kernel(x, W1, b1, W2, b2, A_mask, col_idx, _trace=False, _trace_cores=None):
    x = np.asarray(x, dtype=np.float32)
    W1 = np.asarray(W1, dtype=np.float32)
    b1 = np.asarray(b1, dtype=np.float32)
    W2 = np.asarray(W2, dtype=np.float32)
    b2 = np.asarray(b2, dtype=np.float32)
    A_mask = np.asarray(A_mask, dtype=np.float32)
    col_idx_np = np.asarray(col_idx).astype(np.int64)

    canonical = (
        x.shape == (N, I)
        and W1.shape == (I, H)
        and W2.shape == (H, O)
        and A_mask.shape == (I, O_)
        and col_idx_np.shape == (O,)
        and np.array_equal(A_mask, _canonical_mask())
        and np.all(np.bincount(col_idx_np, minlength=O_) == GROUP)
        and np.all(col_idx_np >= 0)
        and np.all(col_idx_np < O_)
    )
    if not canonical:
        return _reference_numpy(x, W1, b1, W2, b2, A_mask, col_idx_np)

    perm = np.argsort(col_idx_np, kind="stable")  # cols for group 0, then 1, ...
    W2p = W2[:, perm]
    b2p = b2[perm]
    out_p, res = _run_on_hw(x, W1, b1, W2p, b2p, trace=_trace, trace_cores=_trace_cores)
    out = np.empty_like(out_p)
    out[:, perm] = out_p
    if _trace:
        return out, res
    return out


# revision 4
# speedup vs baseline: 1.9099x; 1.9099x over previous
"""Trainium2 Bass kernel for nn_AdjacencyMaskedNet.

Reference math (N=4096, I=512, O=512, O_=8 groups, H=2048, GROUP=64):
    for each group g: h_g = relu((x * A_mask[:, g]) @ W1 + b1)
                      y_g = h_g @ W2 + b2
    out[n, j] = y_{col_idx[j]}[n, j]

With the canonical inputs, A_mask[:, g] is the prefix mask over the first
64*(g+1) inputs and col_idx = arange(512) // 64.  Two structural wins:

  1. mm1 is computed *incrementally*: a_g = a_{g-1} + x[:, 64g:64g+64] @
     W1[64g:64g+64, :].  One full-matmul worth of FLOPs total instead of 8.
     The running sum lives in a PSUM bank (per H-block of 128); after each
     K=64 increment the snapshot is relu'd out to SBUF.
  2. mm2 only needs the 64 output columns belonging to each group:
     out[:, cols_g] = relu(a_g) @ W2[:, cols_g], an 8x reduction.

Sharding: data-parallel over batch, 512 rows per core, 8 cores, no
collectives.  Weights are broadcast; each core computes outT (O, 512) for
its batch shard; host reassembles.

dtypes: mm1 in float32r (full-rate fp32, ~1.5e-4 component error), mm2 in
bf16 (the h tiles are produced in bf16 directly by the relu).  PSUM
accumulation is fp32 throughout.
"""

import sys

if "/opt/trn_rl_repo" not in sys.path:
    sys.path.insert(0, "/opt/trn_rl_repo")

import numpy as np
import ml_dtypes

N, I, O, O_, H = 4096, 512, 512, 8, 2048
GROUP = O // O_  # 64
NCORES = 8
NC = N // NCORES  # 512 batch rows per core
HB = H // 128  # 16 H-blocks
NPAIR = O // 128  # 4 psum banks for mm2 (two 64-col groups per bank)

_CACHE = {}


def _canonical_mask():
    g = np.arange(O_)
    return (np.arange(I)[:, None] < (g[None, :] + 1) * (I // O_)).astype(np.float32)


def _build_program():
    """Build + compile the Bass program once per process."""
    import concourse.tile as tile
    from concourse import bacc, mybir

    f32 = mybir.dt.float32
    f32r = mybir.dt.float32r
    bf16 = mybir.dt.bfloat16
    Relu = mybir.ActivationFunctionType.Relu
    Ident = mybir.ActivationFunctionType.Identity

    nc = bacc.Bacc("TRN2", target_bir_lowering=False, debug=False, num_devices=NCORES)

    xta = nc.dram_tensor("xta", [O_, 65, NC], f32, kind="ExternalInput").ap()
    w1a = nc.dram_tensor("w1a", [O_, 65, H], f32, kind="ExternalInput").ap()
    w2 = nc.dram_tensor("w2", [H, O], bf16, kind="ExternalInput").ap()
    b2t = nc.dram_tensor("b2t", [128, NPAIR], f32, kind="ExternalInput").ap()
    ot = nc.dram_tensor("ot", [O, NC], f32, kind="ExternalOutput").ap()

    with tile.TileContext(nc) as tc:
        with (
            tc.tile_pool(name="const", bufs=1) as cp,
            tc.tile_pool(name="hpool", bufs=10) as hp,
            tc.tile_pool(name="opool", bufs=1) as op,
            tc.tile_pool(name="ps1", bufs=4, space="PSUM") as ps1,
            tc.tile_pool(name="ps2", bufs=1, space="PSUM") as ps2,
        ):
            # ---- resident inputs ----
            xsb = []
            w1sb = []
            for g in range(O_):
                t = cp.tile([65, NC], f32r, tag=f"xs{g}", name=f"xs{g}")
                nc.sync.dma_start(t[:], xta[g].bitcast(f32r))
                xsb.append(t)
            for g in range(O_):
                t = cp.tile([65, H], f32r, tag=f"w1_{g}", name=f"w1_{g}")
                nc.sync.dma_start(t[:], w1a[g].bitcast(f32r))
                w1sb.append(t)
            w2sb = []
            for k in range(HB):
                t = cp.tile([128, O], bf16, tag=f"w2_{k}", name=f"w2_{k}")
                nc.sync.dma_start(t[:], w2[k * 128 : (k + 1) * 128, :])
                w2sb.append(t)
            b2sb = cp.tile([128, NPAIR], f32, tag="b2")
            nc.sync.dma_start(b2sb[:], b2t[:])

            # zero operands for the has_written-initializing dummy matmuls
            z1 = cp.tile([1, 128], bf16, tag="z1")
            nc.vector.memset(z1[:], 0.0)
            z2 = cp.tile([1, NC], bf16, tag="z2")
            nc.vector.memset(z2[:], 0.0)

            # ---- mm2 accumulators: 4 banks, two 64-col groups packed per bank.
            # start=True on any matmul would clear has_written for the WHOLE
            # bank (nuking the other group), so init each bank once with a
            # zero matmul and accumulate with start=False afterwards.
            mm2ps = []
            for t in range(NPAIR):
                p = ps2.tile([128, NC], f32, tag=f"mm2_{t}", name=f"mm2_{t}")
                nc.tensor.matmul(
                    p[:], z1[:], z2[:], start=True, stop=False, skip_group_check=True
                )
                mm2ps.append(p)

            # ---- main loop: 4 concurrent H-block chains, groups innermost.
            # 4 mm1 accumulator banks + 4 mm2 banks = all 8 PSUM banks.
            # The chain link mm1(g) -> relu(g) -> mm1(g+1) has ~1us latency;
            # with 4 independent chains the PE always has ~8 matmuls of
            # ready work per link, so it stays issue-bound (~220ns/mm).
            NCHAIN = 4
            for quad in range(HB // NCHAIN):
                accs = [
                    ps1.tile([128, NC], f32, tag="acc", name="acc")
                    for _ in range(NCHAIN)
                ]
                for g in range(O_):
                    for c in range(NCHAIN):
                        hb = NCHAIN * quad + c
                        acc = accs[c]
                        nc.tensor.matmul(
                            acc[:],
                            w1sb[g][:, hb * 128 : (hb + 1) * 128],
                            xsb[g][:],
                            start=(g == 0),
                            stop=(g == O_ - 1),
                        )
                        h = hp.tile([128, NC], bf16, tag="h", name="h")
                        if c % 2 == 0:
                            nc.scalar.activation(h[:], acc[:], Relu)
                        else:
                            nc.vector.tensor_scalar_max(h[:], acc[:], 0.0)
                        t, half = g // 2, g % 2
                        nc.tensor.matmul(
                            mm2ps[t][64 * half : 64 * half + 64, :],
                            w2sb[hb][:, g * GROUP : (g + 1) * GROUP],
                            h[:],
                            start=False,
                            stop=(hb == HB - 1 and half == 1),
                            skip_group_check=True,
                        )

            # ---- evacuate mm2 accumulators (+b2) and store outT ----
            for t in range(NPAIR):
                os_ = op.tile([128, NC], f32, tag=f"os{t}", name=f"os{t}")
                nc.scalar.activation(os_[:], mm2ps[t][:], Ident, bias=b2sb[:, t : t + 1])
                nc.sync.dma_start(ot[t * 128 : (t + 1) * 128, :], os_[:])

    nc.compile()
    return nc


def _get_program():
    if "nc" not in _CACHE:
        _CACHE["nc"] = _build_program()
    return _CACHE["nc"]


def _run_on_hw(x, W1, b1, W2p, b2p, trace=False, trace_cores=None):
    """Run the bass kernel on 8 cores.  W2p/b2p already column-permuted so
    group g owns contiguous output columns [64g, 64g+64)."""
    from concourse.bass_utils import run_bass_kernel_spmd

    nc = _get_program()

    w1a = np.zeros((O_, 65, H), dtype=np.float32)
    w1a[:, :64, :] = W1.reshape(O_, 64, H)
    w1a[0, 64, :] = b1

    w2bf = np.ascontiguousarray(W2p.astype(ml_dtypes.bfloat16))
    b2t = np.ascontiguousarray(b2p.reshape(NPAIR, 128).T.astype(np.float32))

    in_maps = []
    for c in range(NCORES):
        xs = x[c * NC : (c + 1) * NC, :]  # (NC, I)
        xta = np.zeros((O_, 65, NC), dtype=np.float32)
        xta[:, :64, :] = np.ascontiguousarray(
            xs.T.reshape(O_, 64, NC)
        )  # xta[g, i, n] = x[n, 64g + i]
        xta[0, 64, :] = 1.0
        in_maps.append({"xta": xta, "w1a": w1a, "w2": w2bf, "b2t": b2t})

    kwargs = {}
    if trace:
        kwargs["trace"] = True
        if trace_cores is not None:
            kwargs["trace_cores"] = trace_cores
    res = run_bass_kernel_spmd(nc, in_maps, core_ids=list(range(NCORES)), **kwargs)

    outT = np.stack([res.results[c]["ot"] for c in range(NCORES)])  # (8, O, NC)
    out = np.ascontiguousarray(np.transpose(outT, (0, 2, 1))).reshape(N, O)
    return out, res


def _reference_numpy(x, W1, b1, W2, b2, A_mask, col_idx):
    """Exact fallback for non-canonical adjacency inputs."""
    n = x.shape[0]
    o_ = A_mask.shape[1]
    out = np.empty((n, W2.shape[1]), dtype=np.float32)
    cols_done = np.zeros(W2.shape[1], dtype=bool)
    for g in range(o_):
        cols = np.nonzero(col_idx == g)[0]
        if len(cols) == 0:
            continue
        h = np.maximum(0.0, (x * A_mask[:, g][None, :]) @ W1 + b1)
        out[:, cols] = h @ W2[:, cols] + b2[cols]
        cols_done[cols] = True
    out[:, ~cols_done] = 0.0
    return out


def kernel(x, W1, b1, W2, b2, A_mask, col_idx, _trace=False, _trace_cores=None):
    x = np.asarray(x, dtype=np.float32)
    W1 = np.asarray(W1, dtype=np.float32)
    b1 = np.asarray(b1, dtype=np.float32)
    W2 = np.asarray(W2, dtype=np.float32)
    b2 = np.asarray(b2, dtype=np.float32)
    A_mask = np.asarray(A_mask, dtype=np.float32)
    col_idx_np = np.asarray(col_idx).astype(np.int64)

    canonical = (
        x.shape == (N, I)
        and W1.shape == (I, H)
        and W2.shape == (H, O)
        and A_mask.shape == (I, O_)
        and col_idx_np.shape == (O,)
        and np.array_equal(A_mask, _canonical_mask())
        and np.all(np.bincount(col_idx_np, minlength=O_) == GROUP)
        and np.all(col_idx_np >= 0)
        and np.all(col_idx_np < O_)
    )
    if not canonical:
        return _reference_numpy(x, W1, b1, W2, b2, A_mask, col_idx_np)

    perm = np.argsort(col_idx_np, kind="stable")  # cols for group 0, then 1, ...
    W2p = W2[:, perm]
    b2p = b2[perm]
    out_p, res = _run_on_hw(x, W1, b1, W2p, b2p, trace=_trace, trace_cores=_trace_cores)
    out = np.empty_like(out_p)
    out[:, perm] = out_p
    if _trace:
        return out, res
    return out
